# revision 1
# baseline (speedup 1.0000x reference)
"""Self-contained Trainium2 Bass kernel for nn_DecoderMultiHeadedAttention.

Reference computation (B=4, S=1024, D=1024, H=16, DH=64):
    q = split_heads(query @ Wq.T + bq)        k, v likewise
    scores = q k^T / 8 ; masked fill -1e9 where mask==0 ; softmax
    x = merge_heads(softmax @ v) ; out = x @ Wo.T + bo

Sharding over 8 NeuronCores: core c handles batch b=c//2 and head-group
g=c%2 (8 of the 16 heads == 512 of the 1024 d' features).  Each core
computes a partial output projection; the host sums the two partials per
batch and adds bo.  All transposes/slices are done on host (free).

Per-core device program (S=1024, 8 local heads):
  qT  = (Wq_g X_q^T)            [512,1024]  (d'-major; feeds scores lhsT/rhs)
  kT  = (Wk_g X_k^T)            [512,1024]
  v   = (X_v Wv_g^T)            [1024,512]  (s-major; feeds pv lhsT), +ones col
  per head: scoresT[j,i] = k_j . q_i   (PE, K=64, head pairs row-tiled)
            em = exp(scoresT/8) * maskT          (ACT exp, DVE mul, bf16)
            xT_aug[., i] = v_aug^T @ em   -> rows 0:64 = unnorm xT, row 64 = sum(em)
            xT = xT_aug[0:64] * (1/row64)  (DVE recip + DMA bcast + DVE mul)
  out_p = xT^T Wo_g^T   (accumulate K=128 over 4 head-pair tiles)

Softmax note: row-max subtraction is skipped (scores are O(5), exp is safe)
and the mask is applied multiplicatively AFTER exp: p = em / sum(em) equals
the reference softmax(masked scores) exactly in exact arithmetic.
"""

import numpy as np
import ml_dtypes

import concourse.bass as bass
import concourse.mybir as mybir
import concourse.tile as tile
from concourse import bacc
from concourse import bass_utils

B, S, D, H = 4, 1024, 1024, 16
DH = D // H            # 64
HL = 8                 # heads per core
DL = HL * DH           # 512 local d' features
P = 128                # partitions
NT = S // P            # 8 tiles of 128 along s
KT = D // P            # 8 k-tiles along d

F32 = mybir.dt.float32
F32R = mybir.dt.float32r
BF16 = mybir.dt.bfloat16

# Config: dtype of the streamed activations/weights for the q/k projections
# and of the q/k sbuf tensors + scores matmul. "bf16" halves the startup DMA
# (the exp critical path starts ~13us earlier); "f32" is most accurate
# (scores matmul runs as float32r either way).
QK_DTYPE = BF16

LAST_RESULTS = None  # test harness reads profiling info from here


def _r(ap):
    """View an fp32 AP as float32r for full-rate PE matmuls."""
    return ap.bitcast(F32R)


def build_nc(qk_dtype=QK_DTYPE, debug=False):
    nc = bacc.Bacc("TRN2", target_bir_lowering=False, debug=False, num_devices=8)

    qk_np = np.float32 if qk_dtype == F32 else ml_dtypes.bfloat16

    # all inputs host-pre-shuffled to the exact SBUF layout (partition-major)
    # so every load is one linear DMA with maximal descriptors
    xq = nc.dram_tensor("xq_t", [P, KT, S], qk_dtype, kind="ExternalInput")
    xk = nc.dram_tensor("xk_t", [P, KT, S], qk_dtype, kind="ExternalInput")
    xv = nc.dram_tensor("xv_t", [P, KT, S], qk_dtype, kind="ExternalInput")
    mt = nc.dram_tensor("mask_t", [P, NT, S], BF16, kind="ExternalInput")
    wq = nc.dram_tensor("wq_t", [P, KT, DL], qk_dtype, kind="ExternalInput")
    wk = nc.dram_tensor("wk_t", [P, KT, DL], qk_dtype, kind="ExternalInput")
    wv = nc.dram_tensor("wv_t", [P, KT, DL], qk_dtype, kind="ExternalInput")
    wo = nc.dram_tensor("wo_t", [P, 4, S], BF16, kind="ExternalInput")
    out = nc.dram_tensor("out_p", [S, D], F32, kind="ExternalOutput")
    dbg = {}
    if debug:
        for nm, shp, dt_ in (("dbg_qt", [P, S], F32), ("dbg_kt", [P, S], F32),
                             ("dbg_va", [P, HL * DH], F32), ("dbg_em", [P, S], F32),
                             ("dbg_xp", [P, S], F32)):
            dbg[nm] = nc.dram_tensor(nm, shp, dt_, kind="ExternalOutput")

    def mmcast(ap):
        return _r(ap) if ap.dtype == F32 else ap

    with tile.TileContext(nc) as tc:
        with (
            tc.tile_pool(name="win", bufs=1) as win,         # weight tensors
            tc.tile_pool(name="xin", bufs=1) as xin,         # activation tensors
            tc.tile_pool(name="mask", bufs=1) as maskp,      # resident mask
            tc.tile_pool(name="qk", bufs=4) as qkp,          # qT / kT tensors
            tc.tile_pool(name="vaug", bufs=NT) as vaugp,     # v + ones column
            tc.tile_pool(name="em", bufs=20) as emp,         # exp(scores)*mask
            tc.tile_pool(name="xt", bufs=4) as xtp,          # normalized xT pairs
            tc.tile_pool(name="small", bufs=2) as smallp,    # recip rows, bcasts, tmp
            tc.tile_pool(name="wo", bufs=1) as wop,
            tc.tile_pool(name="outs", bufs=2) as outsp,
            tc.tile_pool(name="dram", bufs=2, space="DRAM") as dramp,
            tc.tile_pool(name="ps", bufs=2, space="PSUM") as psp,    # proj+scores
            tc.tile_pool(name="xps", bufs=2, space="PSUM") as xpsp,  # pv accum
        ):
            # ------- input DMA: batched transfers (few sem lanes; q/k weights
            # m0-sliced so the first projection starts after ~4.5MB) ----------
            xq_sb = xin.tile([P, KT, S], qk_dtype, tag="xq", name="xq_sb")
            nc.sync.dma_start(out=xq_sb, in_=xq.ap())
            xk_sb = xin.tile([P, KT, S], qk_dtype, tag="xk", name="xk_sb")
            nc.sync.dma_start(out=xk_sb, in_=xk.ap())
            wq_sb = win.tile([P, KT, DL], qk_dtype, tag="wq", name="wq_sb")
            wk_sb = win.tile([P, KT, DL], qk_dtype, tag="wk", name="wk_sb")
            for w_t, wt_d in ((wq_sb, wq), (wk_sb, wk)):
                nc.sync.dma_start(out=w_t[:, :, 0:P], in_=wt_d.ap()[:, :, 0:P])
            mask_sb = maskp.tile([P, NT, S], BF16, tag="mask", name="mask_sb")
            nc.sync.dma_start(out=mask_sb, in_=mt.ap())
            for w_t, wt_d in ((wq_sb, wq), (wk_sb, wk)):
                nc.sync.dma_start(out=w_t[:, :, P:DL], in_=wt_d.ap()[:, :, P:DL])
            xv_sb = xin.tile([P, KT, S], qk_dtype, tag="xv", name="xv_sb")
            nc.sync.dma_start(out=xv_sb, in_=xv.ap())
            wv_sb = win.tile([P, KT, DL], qk_dtype, tag="wv", name="wv_sb")
            nc.sync.dma_start(out=wv_sb, in_=wv.ap())
            wo_sb = wop.tile([P, 4, S], BF16, tag="wo", name="wo_sb")
            nc.sync.dma_start(out=wo_sb, in_=wo.ap())

            q_sb = [None] * 4
            k_sb = [None] * 4
            v_aug = [None] * NT
            em_tiles = [[None] * NT for _ in range(HL)]
            xpairs = [None] * 4
            xps_cur = {}

            def filler_burst(m, which):
                """One (proj, s-half) of the qT[m]/kT[m] projection: 8 matmuls
                into a 1-bank psum, cast straight into the q/k sbuf tensor."""
                proj_idx, nh = which // 2, which % 2
                w_t = (wq_sb, wk_sb)[proj_idx]
                x_t = (xq_sb, xk_sb)[proj_idx]
                dst = (q_sb, k_sb)[proj_idx]
                fp = psp.tile([P, 512], F32, tag="big", name="fps")
                for k in range(KT):
                    nc.tensor.matmul(
                        fp,
                        lhsT=mmcast(w_t[:, k, m * P:(m + 1) * P]),
                        rhs=mmcast(x_t[:, k, nh * 512:(nh + 1) * 512]),
                        start=(k == 0), stop=(k == KT - 1),
                    )
                if dst[m] is None:
                    dst[m] = qkp.tile([P, S], qk_dtype, tag="qkt", name="qkt")
                nc.vector.tensor_copy(dst[m][:, nh * 512:(nh + 1) * 512], fp)

            def v_chunk(st):
                """projection of v for s-tile `st`, packed into v_aug layout."""
                ps = psp.tile([P, DL], F32, tag="big", name="vps")
                for k in range(KT):
                    nc.tensor.matmul(
                        ps,
                        lhsT=mmcast(xv_sb[:, k, st * P:(st + 1) * P]),
                        rhs=mmcast(wv_sb[:, k, :]),
                        start=(k == 0), stop=(k == KT - 1),
                    )
                # pv lhsT layout [ones | 63 junk | v]: the ones column in
                # position 0 puts the softmax denominator on psum partition 0
                # (reciprocal_approx_fast breaks at base!=0), v in columns
                # 64:128 puts xT at a legal engine base partition (64).
                va = vaugp.tile([P, HL, P + 2], BF16, tag="va")
                nc.vector.memset(va, 1.0)
                nc.vector.tensor_copy(
                    va[:, :, DH:P],
                    ps[:].rearrange("p (h d) -> p h d", h=HL),
                )
                v_aug[st] = va

            def scores(p, j):
                """scoresT + exp + mask for heads 2p,2p+1 (row-tiled K=64)."""
                ps = psp.tile([P, S], F32, tag="big", name="sA")
                ps2 = psp.tile([P, S], F32, tag="big", name="sB")
                for nh in range(2):
                    for hh in range(2):
                        off = hh * DH
                        dst = ps if hh == 0 else ps2
                        nc.tensor.matmul(
                            dst[:, nh * 512:(nh + 1) * 512],
                            lhsT=mmcast(k_sb[p][off:off + DH, j * P:(j + 1) * P]),
                            rhs=mmcast(q_sb[p][off:off + DH, nh * 512:(nh + 1) * 512]),
                            start=True, stop=True,
                        )
                for hh, srcp in ((0, ps), (1, ps2)):
                    h = 2 * p + hh
                    em = emp.tile([P, S], BF16, tag="em")
                    nc.scalar.activation(
                        em, srcp, mybir.ActivationFunctionType.Exp, scale=0.125,
                    )
                    nc.vector.tensor_mul(em, em, mask_sb[:, j, :])
                    em_tiles[h][j] = em

            def pv(p, j):
                """one j-tile of (v_aug^T @ em) for both heads of pair p."""
                if j == 0:
                    xpairs[p] = xtp.tile([P, S], BF16, tag="xpair", name="xpair")
                    xps_cur[p] = (xpsp.tile([P, S], F32, tag="xps", name="xpsA"),
                                  xpsp.tile([P, S], F32, tag="xps", name="xpsB"))
                for hh in range(2):
                    h = 2 * p + hh
                    xps = xps_cur[p][hh]
                    for nh in range(2):
                        nc.tensor.matmul(
                            xps[:, nh * 512:(nh + 1) * 512],
                            lhsT=v_aug[j][:, h, 0:P],
                            rhs=em_tiles[h][j][:, nh * 512:(nh + 1) * 512],
                            start=(j == 0), stop=(j == NT - 1),
                        )

            def norm(p):
                """xT/sum(em): row 0 of xps = denominator, rows 64:128 = xT.
                Copy out of psum first so the psum slots free fast, then
                multiply in place."""
                xpair = xpairs[p]
                for hh in range(2):
                    xps = xps_cur[p][hh]
                    if hh == 1:
                        dst = xpair
                    else:
                        dst = smallp.tile([P, S], BF16, tag="tmp")
                    nc.vector.tensor_copy(dst[DH:P, :], xps[DH:P, :])
                    r = smallp.tile([1, S], F32, tag="r")
                    nc.vector.reciprocal_approx_fast(out=r, in_=xps[0:1, :])
                    # partition-broadcast via DRAM bounce: engine APs need a
                    # nonzero partition step; a step-0 source is DMA+DRAM-only
                    rd = dramp.tile([1, S], F32, tag="rd")
                    nc.sync.dma_start(out=rd, in_=r)
                    rb = smallp.tile([P, S], F32, tag="rb")
                    nc.sync.dma_start(out=rb[DH:P, :], in_=rd.to_broadcast((DH, S)))
                    nc.vector.tensor_mul(dst[DH:P, :], dst[DH:P, :], rb[DH:P, :])
                    if hh == 0:
                        # DVE cannot shift partitions; DMA moves head A down
                        nc.sync.dma_start(out=xpair[0:DH, :], in_=dst[DH:P, :])

            # ---------------- software-pipelined emission --------------------
            # PE is in-order: inside each iteration, emit work whose inputs
            # are long-ready (pv of the previous pair, projection filler)
            # before the scores matmuls that wait on a psum slot freed by the
            # exp of the previous iteration.  ACT (softmax exp) is the pacing
            # engine; everything else hides behind it.
            for which in range(4):              # qT[0]/kT[0] up front
                filler_burst(0, which)
            for p in range(4):
                for j in range(NT):
                    if p == 0:
                        v_chunk(j)
                    elif p < 3:
                        pv(p - 1, j)
                        if j == NT - 1:
                            norm(p - 1)
                    else:
                        if j < 4:
                            pv(2, 2 * j)
                            pv(2, 2 * j + 1)
                            if j == 3:
                                norm(2)
                        else:
                            sched = {4: (0, 1), 5: (2, 3), 6: (4,), 7: (5,)}
                            for jj in sched[j]:
                                pv(3, jj)
                    if p < 3 and j % 2 == 1:
                        filler_burst(p + 1, (j - 1) // 2)
                    scores(p, j)

            pv(3, 6)
            pv(3, 7)
            norm(3)

            for mtile in range(NT):
                ps = psp.tile([P, S], F32, tag="big", name="ops")
                for nh in range(2):
                    for kp in range(4):
                        nc.tensor.matmul(
                            ps[:, nh * 512:(nh + 1) * 512],
                            lhsT=xpairs[kp][:, mtile * P:(mtile + 1) * P],
                            rhs=wo_sb[:, kp, nh * 512:(nh + 1) * 512],
                            start=(kp == 0), stop=(kp == 3),
                        )
                ob = outsp.tile([P, S], F32, tag="ob", name="ob")
                nc.vector.tensor_copy(ob, ps)
                nc.sync.dma_start(out=out.ap()[mtile * P:(mtile + 1) * P, :], in_=ob)

    nc.compile()
    return nc


def kernel(query, key, value, mask, Wq, bq, Wk, bk, Wv, bv, Wo, bo, **_ignored):
    global LAST_RESULTS
    query = np.asarray(query, np.float32)
    key = np.asarray(key, np.float32)
    value = np.asarray(value, np.float32)
    mask = np.asarray(mask)
    Wq, Wk, Wv, Wo = (np.asarray(w, np.float32) for w in (Wq, Wk, Wv, Wo))
    bq, bk, bv, bo = (np.asarray(b_, np.float32) for b_ in (bq, bk, bv, bo))
    assert not (np.any(bq) or np.any(bk) or np.any(bv)), (
        "kernel assumes zero q/k/v projection biases (true for this problem)"
    )

    qk_np = np.float32 if QK_DTYPE == F32 else ml_dtypes.bfloat16
    WqT, WkT, WvT = Wq.T, Wk.T, Wv.T          # [d, d']
    WoT = np.ascontiguousarray(Wo.T)          # [d', dout]
    mbin = (mask != 0)

    def pmaj(a, chunks):
        """[C*P, W] -> [P, C, W]: partition-major layout for linear DMA."""
        return np.ascontiguousarray(a.reshape(chunks, P, -1).transpose(1, 0, 2))

    in_maps = []
    for c in range(8):
        b, g = c // 2, c % 2
        sl = slice(g * DL, (g + 1) * DL)
        in_maps.append({
            "xq_t": pmaj(np.ascontiguousarray(query[b].T).astype(qk_np), KT),
            "xk_t": pmaj(np.ascontiguousarray(key[b].T).astype(qk_np), KT),
            "xv_t": pmaj(np.ascontiguousarray(value[b].T).astype(qk_np), KT),
            "mask_t": pmaj(np.ascontiguousarray(mbin[b].T).astype(ml_dtypes.bfloat16), NT),
            "wq_t": pmaj(np.ascontiguousarray(WqT[:, sl]).astype(qk_np), KT),
            "wk_t": pmaj(np.ascontiguousarray(WkT[:, sl]).astype(qk_np), KT),
            "wv_t": pmaj(np.ascontiguousarray(WvT[:, sl]).astype(qk_np), KT),
            "wo_t": pmaj(np.ascontiguousarray(WoT[sl, :]).astype(ml_dtypes.bfloat16), 4),
        })

    nc = build_nc()
    res = bass_utils.run_bass_kernel_spmd(nc, in_maps, core_ids=list(range(8)))
    LAST_RESULTS = res
    parts = [r["out_p"] for r in res.results]
    out = np.stack([parts[2 * b] + parts[2 * b + 1] + bo for b in range(B)])
    return out.astype(np.float32)



# revision 6
# speedup vs baseline: 1.1224x; 1.1224x over previous
"""Self-contained Trainium2 Bass kernel for nn_DecoderMultiHeadedAttention.

Reference computation (B=4, S=1024, D=1024, H=16, DH=64):
    q = split_heads(query @ Wq.T + bq)        k, v likewise
    scores = q k^T / 8 ; masked fill -1e9 where mask==0 ; softmax
    x = merge_heads(softmax @ v) ; out = x @ Wo.T + bo

Sharding over 8 NeuronCores: core c handles batch b=c//2 and head-group
g=c%2 (8 of the 16 heads == 512 of the 1024 d' features).  Each core
computes a partial output projection; the host sums the two partials per
batch and adds bo.  All transposes/slices are done on host (free).

Device schedule (three phases, PE-order == emission order):

Phase 1 (projections, k-streamed): all 8 psum banks hold projection
  accumulators; the contraction (k) loop is OUTERMOST so each matmul
  consumes exactly the wq/xq k-slices most recently DMA'd.  DMAs are
  emitted in consumption order, so the first matmul fires ~1.5us in.
    1a: qT = Wq_g X_q^T  (8 accum slots = 4 tiles x 2 halves), cast bf16
    1b: kT likewise
    1c: v   = X_v Wv_g^T (8 s-chunks), packed into v_aug = [ones | v]

Phase 2 (scores+softmax+pv): psum = 2 scores tiles (4 banks) + 2 pv
  accumulators (4 banks).  Per step (p, j): scoresT via PE (K=64 head
  pairs), ACT exp (pacing engine), DVE mask mul, and the pv matmuls of
  3 steps ago (lag keeps em/psum dependencies off the critical path).
    pv psum layout per head: rows 0:64 = sum(em) (64 ones columns in
    lhsT), rows 64:128 = unnormalized xT.
  norm(p): 64-wide reciprocal of psum rows 0:64, DMA partition-shift of
  the recip to rows 64:128, one DVE mul psum*recip -> xpair (bf16).

Phase 3: last pv steps, norm(3), out projection (accumulate K=128 over
  4 head pairs), ACT copies psum->sbuf, DMA out.

Softmax note: row-max subtraction is skipped (scores are O(5), exp is
safe) and the mask is applied multiplicatively AFTER exp: p = em/sum(em)
equals the reference softmax of masked scores exactly in exact math.
"""

import numpy as np
import ml_dtypes

import concourse.bass as bass
import concourse.mybir as mybir
import concourse.tile as tile
from concourse import bacc
from concourse import bass_utils

B, S, D, H = 4, 1024, 1024, 16
DH = D // H            # 64
HL = 8                 # heads per core
DL = HL * DH           # 512 local d' features
P = 128                # partitions
NT = S // P            # 8 tiles of 128 along s
KT = D // P            # 8 k-tiles along d

F32 = mybir.dt.float32
BF16 = mybir.dt.bfloat16

LAST_RESULTS = None  # test harness reads profiling info from here


def build_nc(debug=False):
    nc = bacc.Bacc("TRN2", target_bir_lowering=False, debug=False, num_devices=8)

    # all inputs host-pre-shuffled to the exact SBUF layout (partition-major)
    # so every load is one linear DMA with maximal descriptors
    xq = nc.dram_tensor("xq_t", [P, KT, S], BF16, kind="ExternalInput")
    xk = nc.dram_tensor("xk_t", [P, KT, S], BF16, kind="ExternalInput")
    xv = nc.dram_tensor("xv_t", [P, KT, S], BF16, kind="ExternalInput")
    mt = nc.dram_tensor("mask_t", [P, NT, S], BF16, kind="ExternalInput")
    wq = nc.dram_tensor("wq_t", [P, KT, DL], BF16, kind="ExternalInput")
    wk = nc.dram_tensor("wk_t", [P, KT, DL], BF16, kind="ExternalInput")
    wv = nc.dram_tensor("wv_t", [P, KT, DL], BF16, kind="ExternalInput")
    wo = nc.dram_tensor("wo_t", [P, 4, S], BF16, kind="ExternalInput")
    out = nc.dram_tensor("out_p", [S, D], F32, kind="ExternalOutput")

    with tile.TileContext(nc) as tc:
        with (
            tc.tile_pool(name="win", bufs=1) as win,         # weight tensors
            tc.tile_pool(name="xin", bufs=1) as xin,         # activation tensors
            tc.tile_pool(name="mask", bufs=1) as maskp,      # resident mask
            tc.tile_pool(name="qk", bufs=4) as qkp,          # qT / kT tensors
            tc.tile_pool(name="vaug", bufs=NT) as vaugp,     # [ones | v] lhsT
            tc.tile_pool(name="em", bufs=12) as emp,         # exp(scores)*mask
            tc.tile_pool(name="xt", bufs=4) as xtp,          # normalized xT pairs
            tc.tile_pool(name="rr", bufs=2) as rrp,          # reciprocal rows
            tc.tile_pool(name="r64", bufs=2) as r64p,        # shifted recips
            tc.tile_pool(name="tmpa", bufs=2) as tmpap,      # head-A staging
            tc.tile_pool(name="wo", bufs=1) as wop,
            tc.tile_pool(name="outs", bufs=2) as outsp,
            tc.tile_pool(name="psA", bufs=2, space="PSUM") as psA,   # proj/scores/out
            tc.tile_pool(name="psB", bufs=2, space="PSUM") as psB,   # proj/pv accum
        ):
            # ---------------- input DMAs in consumption order ----------------
            wq_sb = win.tile([P, KT, DL], BF16, tag="wq", name="wq_sb")
            xq_sb = xin.tile([P, KT, S], BF16, tag="xq", name="xq_sb")
            wk_sb = win.tile([P, KT, DL], BF16, tag="wk", name="wk_sb")
            xk_sb = xin.tile([P, KT, S], BF16, tag="xk", name="xk_sb")
            wv_sb = win.tile([P, KT, DL], BF16, tag="wv", name="wv_sb")
            xv_sb = xin.tile([P, KT, S], BF16, tag="xv", name="xv_sb")
            for k in range(KT):
                nc.sync.dma_start(out=wq_sb[:, k, :], in_=wq.ap()[:, k, :])
                nc.sync.dma_start(out=xq_sb[:, k, :], in_=xq.ap()[:, k, :])
            for k in range(KT):
                nc.sync.dma_start(out=wk_sb[:, k, :], in_=wk.ap()[:, k, :])
                nc.sync.dma_start(out=xk_sb[:, k, :], in_=xk.ap()[:, k, :])
            for k in range(KT):
                nc.sync.dma_start(out=wv_sb[:, k, :], in_=wv.ap()[:, k, :])
                nc.sync.dma_start(out=xv_sb[:, k, :], in_=xv.ap()[:, k, :])
            mask_sb = maskp.tile([P, NT, S], BF16, tag="mask", name="mask_sb")
            for j in range(NT):
                nc.sync.dma_start(out=mask_sb[:, j, :], in_=mt.ap()[:, j, :])
            wo_sb = wop.tile([P, 4, S], BF16, tag="wo", name="wo_sb")
            nc.sync.dma_start(out=wo_sb, in_=wo.ap())

            # v_aug ones template: cols 0:64 of each head's lhsT block are 1.0
            # (denominator rows), cols 64:128 get v.  memset everything once
            # up front (runs during the initial DMA wait).
            v_aug = []
            for st in range(NT):
                va = vaugp.tile([P, HL, P + 2], BF16, tag="va")
                nc.vector.memset(va, 1.0)
                v_aug.append(va)

            # ---------------- phase 1: projections, k-streamed ---------------
            q_sb = [None] * 4
            k_sb = [None] * 4

            def proj_qk(w_t, x_t, dst, tag):
                """dst[m][:, :] = (W X^T) rows m*128:(m+1)*128, k-streamed.
                8 accumulation slots live in 4 [P,S] psum tiles (2 pools x 2
                bufs); slot (m, nh) = tile[m][:, nh*512:]."""
                ptiles = [psA.tile([P, S], F32, tag="big", name=f"pp{m}")
                          if m < 2 else
                          psB.tile([P, S], F32, tag="xps", name=f"pp{m}")
                          for m in range(4)]
                for k in range(KT):
                    for m in range(4):
                        for nh in range(2):
                            nc.tensor.matmul(
                                ptiles[m][:, nh * 512:(nh + 1) * 512],
                                lhsT=w_t[:, k, m * P:(m + 1) * P],
                                rhs=x_t[:, k, nh * 512:(nh + 1) * 512],
                                start=(k == 0), stop=(k == KT - 1),
                            )
                for m in range(4):
                    dst[m] = qkp.tile([P, S], BF16, tag=tag, name=tag)
                    nc.vector.tensor_copy(dst[m], ptiles[m])

            proj_qk(wq_sb, xq_sb, q_sb, "qt")
            proj_qk(wk_sb, xk_sb, k_sb, "kt")

            # v projection: slot st = tile[st//2][:, (st%2)*512:], k-streamed
            vtiles = [psA.tile([P, S], F32, tag="big", name=f"vp{i}")
                      if i < 2 else
                      psB.tile([P, S], F32, tag="xps", name=f"vp{i}")
                      for i in range(4)]
            for k in range(KT):
                for st in range(NT):
                    nc.tensor.matmul(
                        vtiles[st // 2][:, (st % 2) * 512:(st % 2 + 1) * 512],
                        lhsT=xv_sb[:, k, st * P:(st + 1) * P],
                        rhs=wv_sb[:, k, :],
                        start=(k == 0), stop=(k == KT - 1),
                    )
            for st in range(NT):
                nc.vector.tensor_copy(
                    v_aug[st][:, :, DH:P],
                    vtiles[st // 2][:, (st % 2) * 512:(st % 2 + 1) * 512]
                        .rearrange("p (h d) -> p h d", h=HL),
                )

            # ---------------- phase 2: scores / softmax / pv -----------------
            em_tiles = [[None] * NT for _ in range(HL)]
            xpairs = [None] * 4
            xps_cur = {}

            def scores(p, j):
                """scoresT + exp + mask for heads 2p,2p+1 (row-tiled K=64)."""
                ps = psA.tile([P, S], F32, tag="big", name="sA")
                ps2 = psA.tile([P, S], F32, tag="big", name="sB")
                for nh in range(2):
                    for hh in range(2):
                        off = hh * DH
                        dst = ps if hh == 0 else ps2
                        nc.tensor.matmul(
                            dst[:, nh * 512:(nh + 1) * 512],
                            lhsT=k_sb[p][off:off + DH, j * P:(j + 1) * P],
                            rhs=q_sb[p][off:off + DH, nh * 512:(nh + 1) * 512],
                            start=True, stop=True,
                        )
                for hh, srcp in ((0, ps), (1, ps2)):
                    h = 2 * p + hh
                    em = emp.tile([P, S], BF16, tag="em")
                    nc.scalar.activation(
                        em, srcp, mybir.ActivationFunctionType.Exp, scale=0.125,
                    )
                    nc.vector.tensor_mul(em, em, mask_sb[:, j, :])
                    em_tiles[h][j] = em

            def pv(p, j):
                """one j-tile of (v_aug^T @ em) for both heads of pair p.
                psum rows 0:64 = running sum(em), rows 64:128 = xT."""
                if j == 0:
                    xpairs[p] = xtp.tile([P, S], BF16, tag="xpair", name="xpair")
                    xps_cur[p] = (xpsB_tile(), xpsB_tile())
                for hh in range(2):
                    h = 2 * p + hh
                    xps = xps_cur[p][hh]
                    for nh in range(2):
                        nc.tensor.matmul(
                            xps[:, nh * 512:(nh + 1) * 512],
                            lhsT=v_aug[j][:, h, 0:P],
                            rhs=em_tiles[h][j][:, nh * 512:(nh + 1) * 512],
                            start=(j == 0), stop=(j == NT - 1),
                        )

            def xpsB_tile():
                return psB.tile([P, S], F32, tag="xps", name="xps")

            def norm(p):
                """xpair = xT / sum(em) straight out of psum.
                recip (64-wide, base 0) -> DMA shift to partitions 64:128 ->
                one DVE mul per head; head A bounces through tmpa for the
                partition shift down to rows 0:64."""
                xpair = xpairs[p]
                rs, r64s = [], []
                for hh in range(2):
                    xps = xps_cur[p][hh]
                    r = rrp.tile([DH, S], F32, tag="r")
                    nc.vector.reciprocal_approx_fast(out=r, in_=xps[0:DH, :])
                    r64 = r64p.tile([P, S], F32, tag="r64")
                    nc.sync.dma_start(out=r64[DH:P, :], in_=r)
                    rs.append(r)
                    r64s.append(r64)
                ta = tmpap.tile([P, S], BF16, tag="tmpa")
                nc.vector.tensor_mul(ta[DH:P, :], xps_cur[p][0][DH:P, :], r64s[0][DH:P, :])
                nc.vector.tensor_mul(xpair[DH:P, :], xps_cur[p][1][DH:P, :], r64s[1][DH:P, :])
                nc.sync.dma_start(out=xpair[0:DH, :], in_=ta[DH:P, :])

            LAG = 3
            for s in range(32 + LAG):
                p, j = s // NT, s % NT
                # matmuls first (PE queue), scores before lagged pv
                if s < 32:
                    scores(p, j)
                if s >= LAG:
                    sp = s - LAG
                    pv(sp // NT, sp % NT)
                    if sp % NT == NT - 1:
                        norm(sp // NT)

            # ---------------- phase 3: output projection ---------------------
            for mtile in range(NT):
                ps = psA.tile([P, S], F32, tag="big", name="ops")
                for nh in range(2):
                    for kp in range(4):
                        nc.tensor.matmul(
                            ps[:, nh * 512:(nh + 1) * 512],
                            lhsT=xpairs[kp][:, mtile * P:(mtile + 1) * P],
                            rhs=wo_sb[:, kp, nh * 512:(nh + 1) * 512],
                            start=(kp == 0), stop=(kp == 3),
                        )
                ob = outsp.tile([P, S], F32, tag="ob", name="ob")
                nc.scalar.copy(ob, ps)
                nc.sync.dma_start(out=out.ap()[mtile * P:(mtile + 1) * P, :], in_=ob)

    nc.compile()
    return nc


def kernel(query, key, value, mask, Wq, bq, Wk, bk, Wv, bv, Wo, bo, **_ignored):
    global LAST_RESULTS
    query = np.asarray(query, np.float32)
    key = np.asarray(key, np.float32)
    value = np.asarray(value, np.float32)
    mask = np.asarray(mask)
    Wq, Wk, Wv, Wo = (np.asarray(w, np.float32) for w in (Wq, Wk, Wv, Wo))
    bq, bk, bv, bo = (np.asarray(b_, np.float32) for b_ in (bq, bk, bv, bo))
    assert not (np.any(bq) or np.any(bk) or np.any(bv)), (
        "kernel assumes zero q/k/v projection biases (true for this problem)"
    )

    bf16 = ml_dtypes.bfloat16
    WqT, WkT, WvT = Wq.T, Wk.T, Wv.T          # [d, d']
    WoT = np.ascontiguousarray(Wo.T)          # [d', dout]
    mbin = (mask != 0)

    def pmaj(a, chunks):
        """[C*P, W] -> [P, C, W]: partition-major layout for linear DMA."""
        return np.ascontiguousarray(a.reshape(chunks, P, -1).transpose(1, 0, 2))

    in_maps = []
    for c in range(8):
        b, g = c // 2, c % 2
        sl = slice(g * DL, (g + 1) * DL)
        in_maps.append({
            "xq_t": pmaj(np.ascontiguousarray(query[b].T).astype(bf16), KT),
            "xk_t": pmaj(np.ascontiguousarray(key[b].T).astype(bf16), KT),
            "xv_t": pmaj(np.ascontiguousarray(value[b].T).astype(bf16), KT),
            "mask_t": pmaj(np.ascontiguousarray(mbin[b].T).astype(bf16), NT),
            "wq_t": pmaj(np.ascontiguousarray(WqT[:, sl]).astype(bf16), KT),
            "wk_t": pmaj(np.ascontiguousarray(WkT[:, sl]).astype(bf16), KT),
            "wv_t": pmaj(np.ascontiguousarray(WvT[:, sl]).astype(bf16), KT),
            "wo_t": pmaj(np.ascontiguousarray(WoT[sl, :]).astype(bf16), 4),
        })

    nc = build_nc()
    res = bass_utils.run_bass_kernel_spmd(nc, in_maps, core_ids=list(range(8)))
    LAST_RESULTS = res
    parts = [r["out_p"] for r in res.results]
    out = np.stack([parts[2 * b] + parts[2 * b + 1] + bo for b in range(B)])
    return out.astype(np.float32)


# revision 14
# speedup vs baseline: 1.1655x; 1.0384x over previous
"""Self-contained Trainium2 Bass kernel for nn_DecoderMultiHeadedAttention.

Reference computation (B=4, S=1024, D=1024, H=16, DH=64):
    q = split_heads(query @ Wq.T + bq)        k, v likewise
    scores = q k^T / 8 ; masked fill -1e9 where mask==0 ; softmax
    x = merge_heads(softmax @ v) ; out = x @ Wo.T + bo

Sharding over 8 NeuronCores: core c handles batch b=c//2 and head-group
g=c%2 (8 of the 16 heads == 512 of the 1024 d' features).  Each core
computes a partial output projection; the host sums the two partials per
batch and adds bo.  All transposes/slices are done on host (free).

Device schedule (three phases, PE-order == emission order):

Phase 1 (projections, k-streamed): all 8 psum banks hold projection
  accumulators; the contraction (k) loop is OUTERMOST so each matmul
  consumes exactly the wq/xq k-slices most recently DMA'd.  DMAs are
  emitted in consumption order, so the first matmul fires ~1.5us in.
    1a: qT = Wq_g X_q^T  (8 accum slots = 4 tiles x 2 halves), cast bf16
    1b: kT likewise
    1c: v   = X_v Wv_g^T (8 s-chunks), packed into v_aug = [ones | v]

Phase 2 (scores+softmax+pv): psum = 2 scores tiles (4 banks) + 2 pv
  accumulators (4 banks).  Per step (p, j): scoresT via PE (K=64 head
  pairs), ACT exp (pacing engine), DVE mask mul, and the pv matmuls of
  3 steps ago (lag keeps em/psum dependencies off the critical path).
    pv psum layout per head: rows 0:64 = sum(em) (64 ones columns in
    lhsT), rows 64:128 = unnormalized xT.
  norm(p): 64-wide reciprocal of psum rows 0:64, DMA partition-shift of
  the recip to rows 64:128, one DVE mul psum*recip -> xpair (bf16).

Phase 3: last pv steps, norm(3), out projection (accumulate K=128 over
  4 head pairs), ACT copies psum->sbuf, DMA out.

Softmax note: row-max subtraction is skipped (scores are O(5), exp is
safe) and the mask is applied multiplicatively AFTER exp: p = em/sum(em)
equals the reference softmax of masked scores exactly in exact math.
"""

import numpy as np
import ml_dtypes

import concourse.bass as bass
import concourse.mybir as mybir
import concourse.tile as tile
from concourse import bacc
from concourse import bass_utils

B, S, D, H = 4, 1024, 1024, 16
DH = D // H            # 64
HL = 8                 # heads per core
DL = HL * DH           # 512 local d' features
P = 128                # partitions
NT = S // P            # 8 tiles of 128 along s
KT = D // P            # 8 k-tiles along d

F32 = mybir.dt.float32
BF16 = mybir.dt.bfloat16

LAST_RESULTS = None  # test harness reads profiling info from here


def build_nc(debug=False):
    nc = bacc.Bacc("TRN2", target_bir_lowering=False, debug=False, num_devices=8)

    # all inputs host-pre-shuffled to the exact SBUF layout (partition-major)
    # so every load is one linear DMA with maximal descriptors
    xq = nc.dram_tensor("xq_t", [P, KT, S], BF16, kind="ExternalInput")
    xk = nc.dram_tensor("xk_t", [P, KT, S], BF16, kind="ExternalInput")
    xv = nc.dram_tensor("xv_t", [P, KT, S], BF16, kind="ExternalInput")
    mt = nc.dram_tensor("mask_t", [P, NT, S], BF16, kind="ExternalInput")
    wq = nc.dram_tensor("wq_t", [P, KT, DL], BF16, kind="ExternalInput")
    wk = nc.dram_tensor("wk_t", [P, KT, DL], BF16, kind="ExternalInput")
    wv = nc.dram_tensor("wv_t", [P, KT, DL], BF16, kind="ExternalInput")
    wo = nc.dram_tensor("wo_t", [P, 4, S], BF16, kind="ExternalInput")
    out = nc.dram_tensor("out_p", [S, D], BF16, kind="ExternalOutput")

    with tile.TileContext(nc) as tc:
        with (
            tc.tile_pool(name="win", bufs=1) as win,         # weight tensors
            tc.tile_pool(name="xin", bufs=1) as xin,         # activation tensors
            tc.tile_pool(name="mask", bufs=1) as maskp,      # resident mask
            tc.tile_pool(name="qk", bufs=4) as qkp,          # qT / kT tensors
            tc.tile_pool(name="vaug", bufs=NT) as vaugp,     # [ones | v] lhsT
            tc.tile_pool(name="em", bufs=14) as emp,         # exp(scores)*mask
            tc.tile_pool(name="xt", bufs=4) as xtp,          # normalized xT pairs
            tc.tile_pool(name="rr", bufs=2) as rrp,          # reciprocal rows
            tc.tile_pool(name="r64", bufs=2) as r64p,        # shifted recips
            tc.tile_pool(name="tmpa", bufs=2) as tmpap,      # head-A staging
            tc.tile_pool(name="wo", bufs=1) as wop,
            tc.tile_pool(name="outs", bufs=2) as outsp,
            tc.tile_pool(name="psA", bufs=2, space="PSUM") as psA,   # proj/scores/out
            tc.tile_pool(name="psB", bufs=2, space="PSUM") as psB,   # proj/pv accum
        ):
            # ---------------- input DMAs in consumption order ----------------
            wq_sb = win.tile([P, KT, DL], BF16, tag="wq", name="wq_sb")
            xq_sb = xin.tile([P, KT, S], BF16, tag="xq", name="xq_sb")
            wk_sb = win.tile([P, KT, DL], BF16, tag="wk", name="wk_sb")
            xk_sb = xin.tile([P, KT, S], BF16, tag="xk", name="xk_sb")
            wv_sb = win.tile([P, KT, DL], BF16, tag="wv", name="wv_sb")
            xv_sb = xin.tile([P, KT, S], BF16, tag="xv", name="xv_sb")
            for k in range(KT):
                nc.sync.dma_start(out=wq_sb[:, k, :], in_=wq.ap()[:, k, :])
                nc.sync.dma_start(out=xq_sb[:, k, :], in_=xq.ap()[:, k, :])
            for k in range(KT):
                nc.sync.dma_start(out=wk_sb[:, k, :], in_=wk.ap()[:, k, :])
                nc.sync.dma_start(out=xk_sb[:, k, :], in_=xk.ap()[:, k, :])
            for k in range(KT):
                nc.sync.dma_start(out=wv_sb[:, k, :], in_=wv.ap()[:, k, :])
                nc.sync.dma_start(out=xv_sb[:, k, :], in_=xv.ap()[:, k, :])
            mask_sb = maskp.tile([P, NT, S], BF16, tag="mask", name="mask_sb")
            for j in range(NT):
                nc.sync.dma_start(out=mask_sb[:, j, :], in_=mt.ap()[:, j, :])
            wo_sb = wop.tile([P, 4, S], BF16, tag="wo", name="wo_sb")
            nc.sync.dma_start(out=wo_sb, in_=wo.ap())

            # v_aug ones template: cols 0:64 of each head's lhsT block are 1.0
            # (denominator rows), cols 64:128 get v.  memset everything once
            # up front (runs during the initial DMA wait).
            v_aug = []
            for st in range(NT):
                va = vaugp.tile([P, HL, P + 2], BF16, tag="va")
                nc.vector.memset(va, 1.0)
                v_aug.append(va)

            # ---------------- phase 1: projections, k-streamed ---------------
            q_sb = [None] * 4
            k_sb = [None] * 4

            def proj_qk(w_t, x_t, dst, tag):
                """dst[m][:, :] = (W X^T) rows m*128:(m+1)*128, k-streamed.
                8 accumulation slots live in 4 [P,S] psum tiles (2 pools x 2
                bufs); slot (m, nh) = tile[m][:, nh*512:]."""
                ptiles = [psA.tile([P, S], F32, tag="big", name=f"pp{m}")
                          if m < 2 else
                          psB.tile([P, S], F32, tag="xps", name=f"pp{m}")
                          for m in range(4)]
                for k in range(KT):
                    for m in range(4):
                        for nh in range(2):
                            nc.tensor.matmul(
                                ptiles[m][:, nh * 512:(nh + 1) * 512],
                                lhsT=w_t[:, k, m * P:(m + 1) * P],
                                rhs=x_t[:, k, nh * 512:(nh + 1) * 512],
                                start=(k == 0), stop=(k == KT - 1),
                            )
                for m in range(4):
                    dst[m] = qkp.tile([P, S], BF16, tag=tag, name=tag)
                    nc.vector.tensor_copy(dst[m], ptiles[m])

            proj_qk(wq_sb, xq_sb, q_sb, "qt")
            proj_qk(wk_sb, xk_sb, k_sb, "kt")

            # v projection first half (st 0..3): slot st = tile[st//2] half,
            # k-streamed before phase 2.  st 4..7 are deferred into phase 2's
            # PE idle (emitted between scores steps, using psB before pv
            # claims it).
            def v_wave(sts, ktiles):
                for k in range(KT):
                    for st in sts:
                        nc.tensor.matmul(
                            ktiles[(st // 2) % 2][:, (st % 2) * 512:(st % 2 + 1) * 512],
                            lhsT=xv_sb[:, k, st * P:(st + 1) * P],
                            rhs=wv_sb[:, k, :],
                            start=(k == 0), stop=(k == KT - 1),
                        )

            def v_cast(sts, ktiles):
                for st in sts:
                    nc.vector.tensor_copy(
                        v_aug[st][:, :, DH:P],
                        ktiles[(st // 2) % 2][:, (st % 2) * 512:(st % 2 + 1) * 512]
                            .rearrange("p (h d) -> p h d", h=HL),
                    )

            vt1 = [psA.tile([P, S], F32, tag="big", name="vpA"),
                   psB.tile([P, S], F32, tag="xps", name="vpB")]
            v_wave([0, 1, 2, 3], vt1)
            v_cast([0, 1, 2, 3], vt1)

            # ---------------- phase 2: scores / softmax / pv -----------------
            em_tiles = [[None] * NT for _ in range(HL)]
            xpairs = [None] * 4
            xps_cur = {}

            def scores(p, j):
                """scoresT + exp + mask for heads 2p,2p+1 (row-tiled K=64)."""
                ps = psA.tile([P, S], F32, tag="big", name="sA")
                ps2 = psA.tile([P, S], F32, tag="big", name="sB")
                for nh in range(2):
                    for hh in range(2):
                        off = hh * DH
                        dst = ps if hh == 0 else ps2
                        nc.tensor.matmul(
                            dst[:, nh * 512:(nh + 1) * 512],
                            lhsT=k_sb[p][off:off + DH, j * P:(j + 1) * P],
                            rhs=q_sb[p][off:off + DH, nh * 512:(nh + 1) * 512],
                            start=True, stop=True,
                        )
                for hh, srcp in ((0, ps), (1, ps2)):
                    h = 2 * p + hh
                    em = emp.tile([P, S], BF16, tag="em")
                    nc.scalar.activation(
                        em, srcp, mybir.ActivationFunctionType.Exp, scale=0.125,
                    )
                    nc.vector.tensor_mul(em, em, mask_sb[:, j, :])
                    em_tiles[h][j] = em

            def pv(p, j):
                """one j-tile of (v_aug^T @ em) for both heads of pair p.
                psum rows 0:64 = running sum(em), rows 64:128 = xT."""
                if j == 0:
                    xpairs[p] = xtp.tile([P, S], BF16, tag="xpair", name="xpair")
                    xps_cur[p] = (xpsB_tile(), xpsB_tile())
                for hh in range(2):
                    h = 2 * p + hh
                    xps = xps_cur[p][hh]
                    for nh in range(2):
                        nc.tensor.matmul(
                            xps[:, nh * 512:(nh + 1) * 512],
                            lhsT=v_aug[j][:, h, 0:P],
                            rhs=em_tiles[h][j][:, nh * 512:(nh + 1) * 512],
                            start=(j == 0), stop=(j == NT - 1),
                        )

            def xpsB_tile():
                return psB.tile([P, S], F32, tag="xps", name="xps")

            def norm(p):
                """xpair = xT / sum(em) straight out of psum.
                recip (64-wide, base 0) -> DMA shift to partitions 64:128 ->
                one DVE mul per head; head A bounces through tmpa for the
                partition shift down to rows 0:64."""
                xpair = xpairs[p]
                r64s = []
                for hh in range(2):
                    xps = xps_cur[p][hh]
                    r = rrp.tile([DH, S], F32, tag="r")
                    nc.vector.reciprocal_approx_fast(out=r, in_=xps[0:DH, :])
                    r64 = r64p.tile([P, S], F32, tag="r64")
                    nc.sync.dma_start(out=r64[DH:P, :], in_=r)
                    r64s.append(r64)
                ta = tmpap.tile([P, S], BF16, tag="tmpa")
                nc.vector.tensor_mul(ta[DH:P, :], xps_cur[p][0][DH:P, :], r64s[0][DH:P, :])
                nc.vector.tensor_mul(xpair[DH:P, :], xps_cur[p][1][DH:P, :], r64s[1][DH:P, :])
                nc.sync.dma_start(out=xpair[0:DH, :], in_=ta[DH:P, :])

            # deferred second v wave (st 4..7): its 32 matmuls are spread over
            # the first phase-2 steps as PE filler; psB is free until pv(0,0).
            vt2 = [psB.tile([P, S], F32, tag="xps", name="vpA2"),
                   psB.tile([P, S], F32, tag="xps", name="vpB2")]
            V2_SCHED = {0: (0, 1, 2), 1: (3, 4, 5), 2: (6, 7)}

            LAG = 4
            for s in range(32 + LAG):
                p, j = s // NT, s % NT
                # matmuls first (PE queue), scores before lagged pv
                if s < 32:
                    scores(p, j)
                for kv in V2_SCHED.get(s, ()):
                    for st in (4, 5, 6, 7):
                        nc.tensor.matmul(
                            vt2[(st // 2) % 2][:, (st % 2) * 512:(st % 2 + 1) * 512],
                            lhsT=xv_sb[:, kv, st * P:(st + 1) * P],
                            rhs=wv_sb[:, kv, :],
                            start=(kv == 0), stop=(kv == KT - 1),
                        )
                if s == 2:
                    v_cast([4, 5, 6, 7], vt2)
                if s >= LAG:
                    sp = s - LAG
                    pv(sp // NT, sp % NT)
                    if sp % NT == NT - 1:
                        norm(sp // NT)

            # ---------------- phase 3: output projection ---------------------
            for mtile in range(NT):
                ps = psA.tile([P, S], F32, tag="big", name="ops")
                for nh in range(2):
                    for kp in range(4):
                        nc.tensor.matmul(
                            ps[:, nh * 512:(nh + 1) * 512],
                            lhsT=xpairs[kp][:, mtile * P:(mtile + 1) * P],
                            rhs=wo_sb[:, kp, nh * 512:(nh + 1) * 512],
                            start=(kp == 0), stop=(kp == 3),
                        )
                ob = outsp.tile([P, S], BF16, tag="ob", name="ob")
                nc.scalar.copy(ob, ps)
                nc.sync.dma_start(out=out.ap()[mtile * P:(mtile + 1) * P, :], in_=ob)

    nc.compile()
    return nc


def kernel(query, key, value, mask, Wq, bq, Wk, bk, Wv, bv, Wo, bo, **_ignored):
    global LAST_RESULTS
    query = np.asarray(query, np.float32)
    key = np.asarray(key, np.float32)
    value = np.asarray(value, np.float32)
    mask = np.asarray(mask)
    Wq, Wk, Wv, Wo = (np.asarray(w, np.float32) for w in (Wq, Wk, Wv, Wo))
    bq, bk, bv, bo = (np.asarray(b_, np.float32) for b_ in (bq, bk, bv, bo))
    assert not (np.any(bq) or np.any(bk) or np.any(bv)), (
        "kernel assumes zero q/k/v projection biases (true for this problem)"
    )

    bf16 = ml_dtypes.bfloat16
    WqT, WkT, WvT = Wq.T, Wk.T, Wv.T          # [d, d']
    WoT = np.ascontiguousarray(Wo.T)          # [d', dout]
    mbin = (mask != 0)

    def pmaj(a, chunks):
        """[C*P, W] -> [P, C, W]: partition-major layout for linear DMA."""
        return np.ascontiguousarray(a.reshape(chunks, P, -1).transpose(1, 0, 2))

    in_maps = []
    for c in range(8):
        b, g = c // 2, c % 2
        sl = slice(g * DL, (g + 1) * DL)
        in_maps.append({
            "xq_t": pmaj(np.ascontiguousarray(query[b].T).astype(bf16), KT),
            "xk_t": pmaj(np.ascontiguousarray(key[b].T).astype(bf16), KT),
            "xv_t": pmaj(np.ascontiguousarray(value[b].T).astype(bf16), KT),
            "mask_t": pmaj(np.ascontiguousarray(mbin[b].T).astype(bf16), NT),
            "wq_t": pmaj(np.ascontiguousarray(WqT[:, sl]).astype(bf16), KT),
            "wk_t": pmaj(np.ascontiguousarray(WkT[:, sl]).astype(bf16), KT),
            "wv_t": pmaj(np.ascontiguousarray(WvT[:, sl]).astype(bf16), KT),
            "wo_t": pmaj(np.ascontiguousarray(WoT[sl, :]).astype(bf16), 4),
        })

    nc = build_nc()
    res = bass_utils.run_bass_kernel_spmd(nc, in_maps, core_ids=list(range(8)))
    LAST_RESULTS = res
    parts = [np.asarray(r["out_p"], np.float32) for r in res.results]
    out = np.stack([parts[2 * b] + parts[2 * b + 1] + bo for b in range(B)])
    return out.astype(np.float32)


# revision 17
# speedup vs baseline: 1.1691x; 1.0031x over previous
"""Self-contained Trainium2 Bass kernel for nn_DecoderMultiHeadedAttention.

Reference computation (B=4, S=1024, D=1024, H=16, DH=64):
    q = split_heads(query @ Wq.T + bq)        k, v likewise
    scores = q k^T / 8 ; masked fill -1e9 where mask==0 ; softmax
    x = merge_heads(softmax @ v) ; out = x @ Wo.T + bo

Sharding over 8 NeuronCores: core c handles batch b=c//2 and head-group
g=c%2 (8 of the 16 heads == 512 of the 1024 d' features).  Each core
computes a partial output projection; the host sums the two partials per
batch and adds bo.  All transposes/slices are done on host (free).

Device schedule (three phases, PE-order == emission order):

Phase 1 (projections, k-streamed): all 8 psum banks hold projection
  accumulators; the contraction (k) loop is OUTERMOST so each matmul
  consumes exactly the wq/xq k-slices most recently DMA'd.  DMAs are
  emitted in consumption order, so the first matmul fires ~1.5us in.
    1a: qT = Wq_g X_q^T  (8 accum slots = 4 tiles x 2 halves), cast bf16
    1b: kT likewise
    1c: v   = X_v Wv_g^T (8 s-chunks), packed into v_aug = [ones | v]

Phase 2 (scores+softmax+pv): psum = 2 scores tiles (4 banks) + 2 pv
  accumulators (4 banks).  Per step (p, j): scoresT via PE (K=64 head
  pairs), ACT exp (pacing engine), DVE mask mul, and the pv matmuls of
  3 steps ago (lag keeps em/psum dependencies off the critical path).
    pv psum layout per head: rows 0:64 = sum(em) (64 ones columns in
    lhsT), rows 64:128 = unnormalized xT.
  norm(p): 64-wide reciprocal of psum rows 0:64, DMA partition-shift of
  the recip to rows 64:128, one DVE mul psum*recip -> xpair (bf16).

Phase 3: last pv steps, norm(3), out projection (accumulate K=128 over
  4 head pairs), ACT copies psum->sbuf, DMA out.

Softmax note: row-max subtraction is skipped (scores are O(5), exp is
safe) and the mask is applied multiplicatively AFTER exp: p = em/sum(em)
equals the reference softmax of masked scores exactly in exact math.
"""

import numpy as np
import ml_dtypes

import concourse.bass as bass
import concourse.mybir as mybir
import concourse.tile as tile
from concourse import bacc
from concourse import bass_utils

B, S, D, H = 4, 1024, 1024, 16
DH = D // H            # 64
HL = 8                 # heads per core
DL = HL * DH           # 512 local d' features
P = 128                # partitions
NT = S // P            # 8 tiles of 128 along s
KT = D // P            # 8 k-tiles along d

F32 = mybir.dt.float32
BF16 = mybir.dt.bfloat16

LAST_RESULTS = None  # test harness reads profiling info from here


def build_nc(debug=False):
    nc = bacc.Bacc("TRN2", target_bir_lowering=False, debug=False, num_devices=8)

    # all inputs host-pre-shuffled to the exact SBUF layout (partition-major)
    # so every load is one linear DMA with maximal descriptors
    xq = nc.dram_tensor("xq_t", [P, KT, S], BF16, kind="ExternalInput")
    xk = nc.dram_tensor("xk_t", [P, KT, S], BF16, kind="ExternalInput")
    xv = nc.dram_tensor("xv_t", [P, KT, S], BF16, kind="ExternalInput")
    mt = nc.dram_tensor("mask_t", [P, NT, S], BF16, kind="ExternalInput")
    wq = nc.dram_tensor("wq_t", [P, KT, DL], BF16, kind="ExternalInput")
    wk = nc.dram_tensor("wk_t", [P, KT, DL], BF16, kind="ExternalInput")
    wv = nc.dram_tensor("wv_t", [P, KT, DL], BF16, kind="ExternalInput")
    wo = nc.dram_tensor("wo_t", [P, 4, S], BF16, kind="ExternalInput")
    out = nc.dram_tensor("out_p", [S, D], BF16, kind="ExternalOutput")

    with tile.TileContext(nc) as tc:
        with (
            tc.tile_pool(name="win", bufs=1) as win,         # weight tensors
            tc.tile_pool(name="xin", bufs=1) as xin,         # activation tensors
            tc.tile_pool(name="mask", bufs=1) as maskp,      # resident mask
            tc.tile_pool(name="qk", bufs=4) as qkp,          # qT / kT tensors
            tc.tile_pool(name="vaug", bufs=NT) as vaugp,     # [ones | v] lhsT
            tc.tile_pool(name="em", bufs=16) as emp,         # exp(scores)*mask
            tc.tile_pool(name="xt", bufs=4) as xtp,          # normalized xT pairs
            tc.tile_pool(name="rr", bufs=2) as rrp,          # reciprocal rows
            tc.tile_pool(name="r64", bufs=2) as r64p,        # shifted recips
            tc.tile_pool(name="tmpa", bufs=2) as tmpap,      # head-A staging
            tc.tile_pool(name="wo", bufs=1) as wop,
            tc.tile_pool(name="outs", bufs=2) as outsp,
            tc.tile_pool(name="psA", bufs=2, space="PSUM") as psA,   # proj/scores/out
            tc.tile_pool(name="psB", bufs=2, space="PSUM") as psB,   # proj/pv accum
        ):
            # ---------------- input DMAs in consumption order ----------------
            wq_sb = win.tile([P, KT, DL], BF16, tag="wq", name="wq_sb")
            xq_sb = xin.tile([P, KT, S], BF16, tag="xq", name="xq_sb")
            wk_sb = win.tile([P, KT, DL], BF16, tag="wk", name="wk_sb")
            xk_sb = xin.tile([P, KT, S], BF16, tag="xk", name="xk_sb")
            wv_sb = win.tile([P, KT, DL], BF16, tag="wv", name="wv_sb")
            xv_sb = xin.tile([P, KT, S], BF16, tag="xv", name="xv_sb")
            for k in range(KT):
                nc.sync.dma_start(out=wq_sb[:, k, :], in_=wq.ap()[:, k, :])
                nc.sync.dma_start(out=xq_sb[:, k, :], in_=xq.ap()[:, k, :])
            for k in range(KT):
                nc.sync.dma_start(out=wk_sb[:, k, :], in_=wk.ap()[:, k, :])
                nc.sync.dma_start(out=xk_sb[:, k, :], in_=xk.ap()[:, k, :])
            for k in range(KT):
                nc.sync.dma_start(out=wv_sb[:, k, :], in_=wv.ap()[:, k, :])
                nc.sync.dma_start(out=xv_sb[:, k, :], in_=xv.ap()[:, k, :])
            mask_sb = maskp.tile([P, NT, S], BF16, tag="mask", name="mask_sb")
            for j in range(NT):
                nc.sync.dma_start(out=mask_sb[:, j, :], in_=mt.ap()[:, j, :])
            wo_sb = wop.tile([P, 4, S], BF16, tag="wo", name="wo_sb")
            nc.sync.dma_start(out=wo_sb, in_=wo.ap())

            # v_aug ones template: cols 0:64 of each head's lhsT block are 1.0
            # (denominator rows), cols 64:128 get v.  memset everything once
            # up front (runs during the initial DMA wait).
            v_aug = []
            for st in range(NT):
                va = vaugp.tile([P, HL, P + 2], BF16, tag="va")
                nc.vector.memset(va, 1.0)
                v_aug.append(va)

            # ---------------- phase 1: projections, k-streamed ---------------
            q_sb = [None] * 4
            k_sb = [None] * 4

            def proj_qk(w_t, x_t, dst, tag):
                """dst[m][:, :] = (W X^T) rows m*128:(m+1)*128, k-streamed.
                8 accumulation slots live in 4 [P,S] psum tiles (2 pools x 2
                bufs); slot (m, nh) = tile[m][:, nh*512:]."""
                ptiles = [psA.tile([P, S], F32, tag="big", name=f"pp{m}")
                          if m < 2 else
                          psB.tile([P, S], F32, tag="xps", name=f"pp{m}")
                          for m in range(4)]
                for k in range(KT):
                    for m in range(4):
                        for nh in range(2):
                            nc.tensor.matmul(
                                ptiles[m][:, nh * 512:(nh + 1) * 512],
                                lhsT=w_t[:, k, m * P:(m + 1) * P],
                                rhs=x_t[:, k, nh * 512:(nh + 1) * 512],
                                start=(k == 0), stop=(k == KT - 1),
                            )
                for m in range(4):
                    dst[m] = qkp.tile([P, S], BF16, tag=tag, name=tag)
                    nc.vector.tensor_copy(dst[m], ptiles[m])

            proj_qk(wq_sb, xq_sb, q_sb, "qt")
            proj_qk(wk_sb, xk_sb, k_sb, "kt")

            # v projection first half (st 0..3): slot st = tile[st//2] half,
            # k-streamed before phase 2.  st 4..7 are deferred into phase 2's
            # PE idle (emitted between scores steps, using psB before pv
            # claims it).
            def v_wave(sts, ktiles):
                for k in range(KT):
                    for st in sts:
                        nc.tensor.matmul(
                            ktiles[(st // 2) % 2][:, (st % 2) * 512:(st % 2 + 1) * 512],
                            lhsT=xv_sb[:, k, st * P:(st + 1) * P],
                            rhs=wv_sb[:, k, :],
                            start=(k == 0), stop=(k == KT - 1),
                        )

            def v_cast(sts, ktiles):
                for st in sts:
                    nc.vector.tensor_copy(
                        v_aug[st][:, :, DH:P],
                        ktiles[(st // 2) % 2][:, (st % 2) * 512:(st % 2 + 1) * 512]
                            .rearrange("p (h d) -> p h d", h=HL),
                    )

            vt1 = [psA.tile([P, S], F32, tag="big", name="vpA"),
                   psB.tile([P, S], F32, tag="xps", name="vpB")]
            v_wave([0, 1, 2, 3], vt1)
            v_cast([0, 1, 2, 3], vt1)

            # ---------------- phase 2: scores / softmax / pv -----------------
            em_tiles = [[None] * NT for _ in range(HL)]
            xpairs = [None] * 4
            xps_cur = {}

            def scores(p, j):
                """scoresT + exp + mask for heads 2p,2p+1 (row-tiled K=64)."""
                ps = psA.tile([P, S], F32, tag="big", name="sA")
                ps2 = psA.tile([P, S], F32, tag="big", name="sB")
                for nh in range(2):
                    for hh in range(2):
                        off = hh * DH
                        dst = ps if hh == 0 else ps2
                        nc.tensor.matmul(
                            dst[:, nh * 512:(nh + 1) * 512],
                            lhsT=k_sb[p][off:off + DH, j * P:(j + 1) * P],
                            rhs=q_sb[p][off:off + DH, nh * 512:(nh + 1) * 512],
                            start=True, stop=True,
                        )
                for hh, srcp in ((0, ps), (1, ps2)):
                    h = 2 * p + hh
                    em = emp.tile([P, S], BF16, tag="em")
                    nc.scalar.activation(
                        em, srcp, mybir.ActivationFunctionType.Exp, scale=0.125,
                    )
                    # head A's mask mul runs on the otherwise-idle Pool
                    # engine (2.1us/tile, hidden by the pv lag); head B stays
                    # on DVE.  Keeps DVE under ~55% so em never gates pv.
                    eng = nc.gpsimd if hh == 0 else nc.vector
                    eng.tensor_mul(em, em, mask_sb[:, j, :])
                    em_tiles[h][j] = em

            def pv(p, j):
                """one j-tile of (v_aug^T @ em) for both heads of pair p.
                psum rows 0:64 = running sum(em), rows 64:128 = xT."""
                if j == 0:
                    xpairs[p] = xtp.tile([P, S], BF16, tag="xpair", name="xpair")
                    xps_cur[p] = (xpsB_tile(), xpsB_tile())
                for hh in range(2):
                    h = 2 * p + hh
                    xps = xps_cur[p][hh]
                    for nh in range(2):
                        nc.tensor.matmul(
                            xps[:, nh * 512:(nh + 1) * 512],
                            lhsT=v_aug[j][:, h, 0:P],
                            rhs=em_tiles[h][j][:, nh * 512:(nh + 1) * 512],
                            start=(j == 0), stop=(j == NT - 1),
                        )

            def xpsB_tile():
                return psB.tile([P, S], F32, tag="xps", name="xps")

            def norm(p):
                """xpair = xT / sum(em) straight out of psum.
                recip (64-wide, base 0) -> DMA shift to partitions 64:128 ->
                one DVE mul per head; head A bounces through tmpa for the
                partition shift down to rows 0:64.  The last pair is on the
                critical path into the output projection, so its shift/mul/
                shift chain is split into s-halves (mtiles 0..3 unlock after
                the first half)."""
                xpair = xpairs[p]
                halves = (slice(0, 512), slice(512, S)) if p == 3 else (slice(0, S),)
                r64s = []
                for hh in range(2):
                    xps = xps_cur[p][hh]
                    r = rrp.tile([DH, S], F32, tag="r")
                    nc.vector.reciprocal_approx_fast(out=r, in_=xps[0:DH, :])
                    r64 = r64p.tile([P, S], F32, tag="r64")
                    for sl in halves:
                        nc.sync.dma_start(out=r64[DH:P, sl], in_=r[:, sl])
                    r64s.append(r64)
                ta = tmpap.tile([P, S], BF16, tag="tmpa")
                for sl in halves:
                    nc.vector.tensor_mul(ta[DH:P, sl], xps_cur[p][0][DH:P, sl], r64s[0][DH:P, sl])
                    nc.vector.tensor_mul(xpair[DH:P, sl], xps_cur[p][1][DH:P, sl], r64s[1][DH:P, sl])
                    nc.sync.dma_start(out=xpair[0:DH, sl], in_=ta[DH:P, sl])

            # deferred second v wave (st 4..7): its 32 matmuls are spread over
            # the first phase-2 steps as PE filler; psB is free until pv(0,0).
            vt2 = [psB.tile([P, S], F32, tag="xps", name="vpA2"),
                   psB.tile([P, S], F32, tag="xps", name="vpB2")]
            V2_SCHED = {0: (0, 1, 2), 1: (3, 4, 5), 2: (6, 7)}

            LAG = 4
            for s in range(32 + LAG):
                p, j = s // NT, s % NT
                # matmuls first (PE queue), scores before lagged pv
                if s < 32:
                    scores(p, j)
                for kv in V2_SCHED.get(s, ()):
                    for st in (4, 5, 6, 7):
                        nc.tensor.matmul(
                            vt2[(st // 2) % 2][:, (st % 2) * 512:(st % 2 + 1) * 512],
                            lhsT=xv_sb[:, kv, st * P:(st + 1) * P],
                            rhs=wv_sb[:, kv, :],
                            start=(kv == 0), stop=(kv == KT - 1),
                        )
                if s == 2:
                    v_cast([4, 5, 6, 7], vt2)
                if s >= LAG:
                    sp = s - LAG
                    pv(sp // NT, sp % NT)
                    if sp % NT == NT - 1:
                        norm(sp // NT)

            # ---------------- phase 3: output projection ---------------------
            for mtile in range(NT):
                ps = psA.tile([P, S], F32, tag="big", name="ops")
                for nh in range(2):
                    for kp in range(4):
                        nc.tensor.matmul(
                            ps[:, nh * 512:(nh + 1) * 512],
                            lhsT=xpairs[kp][:, mtile * P:(mtile + 1) * P],
                            rhs=wo_sb[:, kp, nh * 512:(nh + 1) * 512],
                            start=(kp == 0), stop=(kp == 3),
                        )
                ob = outsp.tile([P, S], BF16, tag="ob", name="ob")
                nc.scalar.copy(ob, ps)
                nc.sync.dma_start(out=out.ap()[mtile * P:(mtile + 1) * P, :], in_=ob)

    nc.compile()
    return nc


def kernel(query, key, value, mask, Wq, bq, Wk, bk, Wv, bv, Wo, bo, **_ignored):
    global LAST_RESULTS
    query = np.asarray(query, np.float32)
    key = np.asarray(key, np.float32)
    value = np.asarray(value, np.float32)
    mask = np.asarray(mask)
    Wq, Wk, Wv, Wo = (np.asarray(w, np.float32) for w in (Wq, Wk, Wv, Wo))
    bq, bk, bv, bo = (np.asarray(b_, np.float32) for b_ in (bq, bk, bv, bo))
    assert not (np.any(bq) or np.any(bk) or np.any(bv)), (
        "kernel assumes zero q/k/v projection biases (true for this problem)"
    )

    bf16 = ml_dtypes.bfloat16
    WqT, WkT, WvT = Wq.T, Wk.T, Wv.T          # [d, d']
    WoT = np.ascontiguousarray(Wo.T)          # [d', dout]
    mbin = (mask != 0)

    def pmaj(a, chunks):
        """[C*P, W] -> [P, C, W]: partition-major layout for linear DMA."""
        return np.ascontiguousarray(a.reshape(chunks, P, -1).transpose(1, 0, 2))

    in_maps = []
    for c in range(8):
        b, g = c // 2, c % 2
        sl = slice(g * DL, (g + 1) * DL)
        in_maps.append({
            "xq_t": pmaj(np.ascontiguousarray(query[b].T).astype(bf16), KT),
            "xk_t": pmaj(np.ascontiguousarray(key[b].T).astype(bf16), KT),
            "xv_t": pmaj(np.ascontiguousarray(value[b].T).astype(bf16), KT),
            "mask_t": pmaj(np.ascontiguousarray(mbin[b].T).astype(bf16), NT),
            "wq_t": pmaj(np.ascontiguousarray(WqT[:, sl]).astype(bf16), KT),
            "wk_t": pmaj(np.ascontiguousarray(WkT[:, sl]).astype(bf16), KT),
            "wv_t": pmaj(np.ascontiguousarray(WvT[:, sl]).astype(bf16), KT),
            "wo_t": pmaj(np.ascontiguousarray(WoT[sl, :]).astype(bf16), 4),
        })

    nc = build_nc()
    res = bass_utils.run_bass_kernel_spmd(nc, in_maps, core_ids=list(range(8)))
    LAST_RESULTS = res
    parts = [np.asarray(r["out_p"], np.float32) for r in res.results]
    out = np.stack([parts[2 * b] + parts[2 * b + 1] + bo for b in range(B)])
    return out.astype(np.float32)
